# revision 25
# baseline (speedup 1.0000x reference)
"""Trainium2 Bass kernel for nn_Centroid (segment_reduce + EMA).

Computes, for full inputs:
    sums   = segment_sum(embed, y, C)            # [C, D]
    counts = segment_sum(ones,  y, C)            # [C]
    out    = THETA*centroid + (1-THETA) * sums/(counts+EPS)

Sharding strategy (class-sharded, not batch-sharded):
  Core i owns classes [i*125, (i+1)*125). Host computes, per core, the list
  of batch-row indices whose label is owned by that core (pure index logic).
  Each core then:
    1. gathers its ~B/8 embed rows from HBM via chunked dma_gather (each
       full row is read exactly once across all cores -> same HBM traffic
       as a contiguous batch shard),
    2. builds a local one-hot [128 rows x 128 local classes] per k-tile via
       a host-provided iota constant + is_equal,
    3. matmul-accumulates sums [125,1024] and counts [125,2] in PSUM using
       float32r (full-rate fp32 matmul, TF32-like) straight from the
       gathered fp32 data (no bf16 cast stage),
    4. divides by counts, applies the EMA with its centroid slice, and
       writes its 125-row slice of the output.
  No cross-core reduction is needed at all (each class is computed fully on
  one core), so there are no collectives.

Scheduling notes (from trace analysis of the previous version):
  - The gather is SWDGE descriptor-generation / SDMA bound (~436 GB/s
    aggregate); the critical path is gpsimd desc-gen back-to-back.
  - The first gather must not wait on unrelated input DMAs: idx is loaded
    first (sync engine), iota/yloc on the scalar engine, and the large
    centroid load is issued only AFTER all gathers, so it overlaps the
    gather window instead of gating it.
  - All 17 one-hot builds + count matmuls are hoisted before the data
    matmuls (they need no gathered data), so counts/inv are ready long
    before the last data matmul and the tail is just mult,mult,add + store.
  - The output store is split by rows across both HWDGE engines.
"""

import os

import numpy as np

import concourse.bacc as bacc
import concourse.mybir as mybir
import concourse.tile as tile
from concourse import library_config
from concourse.bass_utils import run_bass_kernel_spmd
from concourse.tile_rust import add_dep_helper

NCORES = 8
B = 16384
C = 1000
D = 1024
CPC = C // NCORES  # classes per core = 125
P = 128
THETA = 0.7
EPS = 1e-8
DUMMY = CPC  # local class id used for padding rows; discarded

_NC_CACHE: dict[int, object] = {}

# test.py sets KERNEL_TRACE=1 to collect an NTFF profile; results stashed here.
LAST_RESULTS = None


def _chunk_plan(T: int) -> list[int]:
    """Gather chunk sizes in 128-row tiles.

    SWDGE is store-and-forward per call and the ring drains serialize, so the
    schedule is: gen1 | drain1+gen2 | drain2+gen3 | ... | drain_last | matmul
    tail. A small first chunk starts the drain chain early; small last chunks
    keep the trailing drain+matmul short; big middle chunks amortize the
    ~1.1us fixed desc-gen cost per call.
    """
    if T == 17:
        return [2, 3, 3, 3, 2, 2, 1, 1]  # brute-forced against the timing model
    if T <= 4:
        return [1] * T
    chunks = [2]
    left = T - 2
    while left > 9:
        chunks.append(6)
        left -= 6
    if left > 3:
        chunks.append(left - 3)
        left = 3
    chunks += [2, 1]
    return chunks


def _build_nc(n_pad: int):
    """Build + compile the per-core Bass program for a padded shard of n_pad rows."""
    f32 = mybir.dt.float32
    f32r = mybir.dt.float32r
    i16 = mybir.dt.int16
    T = n_pad // P  # number of 128-row k-tiles
    chunks = _chunk_plan(T)

    nc = bacc.Bacc(
        "TRN2",
        target_bir_lowering=False,
        debug=False,
        enable_asserts=False,
        num_devices=NCORES,
    )
    embed_d = nc.dram_tensor("embed", [B, D], f32, kind="ExternalInput")
    idx_d = nc.dram_tensor("idx", [P, n_pad // 16], i16, kind="ExternalInput")
    yloc_d = nc.dram_tensor("yloc", [P, T], f32, kind="ExternalInput")
    # cent/out padded to 128 partitions: 128-partition HWDGE DMAs spread
    # across all 16 SDMA queues; odd partition counts can land on one queue.
    cent_d = nc.dram_tensor("cent", [P, D], f32, kind="ExternalInput")
    iota_d = nc.dram_tensor("iotac", [P, P], f32, kind="ExternalInput")
    # out stacked as two contiguous [P, 512] halves so each half can be
    # stored as soon as its PSUM bank is done (host concatenates columns).
    out_d = nc.dram_tensor("out", [2, P, 512], f32, kind="ExternalOutput")

    # Trigger the Q7 mlp ucode load as early as possible (main block): the
    # load is asynchronous and the first DMAGatherAnt stalls ~10us on it.
    nc.gpsimd.load_library(library_config.mlp)

    with tile.TileContext(nc) as tc:
        with (
            tc.tile_pool(name="const", bufs=1) as cpool,
            tc.tile_pool(name="gather", bufs=len(chunks)) as gpool,
            tc.tile_pool(name="oh", bufs=T) as ohpool,
            tc.tile_pool(name="psum", bufs=1, space="PSUM") as psum,
        ):
            # No explicit load_library: the framework auto-inserts the mlp
            # reload for DMAGatherAnt; an explicit one doubles the ~6us Q7
            # ucode load on the critical path.

            # idx first (gates the gathers), small constants on the other
            # HWDGE engine so they cannot delay it.
            idx_t = cpool.tile([P, n_pad // 16], i16)
            nc.sync.dma_start(out=idx_t[:], in_=idx_d[:])
            iota_t = cpool.tile([P, P], f32)
            nc.scalar.dma_start(out=iota_t[:], in_=iota_d[:])
            yloc_t = cpool.tile([P, T], f32)
            nc.scalar.dma_start(out=yloc_t[:], in_=yloc_d[:])
            # memset cannot write float32r; synthesize 1.0s via DVE (rounds to f32r)
            ones_t = cpool.tile([P, 2], f32r)
            nc.vector.tensor_scalar(
                out=ones_t[:],
                in0=iota_t[:, 0:2],
                scalar1=0.0,
                scalar2=1.0,
                op0=mybir.AluOpType.mult,
                op1=mybir.AluOpType.add,
            )

            ps0 = psum.tile([P, 512], f32)
            ps1 = psum.tile([P, 512], f32)
            pcnt = psum.tile([P, 2], f32)

            # Issue every gather immediately; the only data dep is idx_t.
            gtiles = []
            row0 = 0
            for ct in chunks:
                ch = ct * P
                g = gpool.tile([P, ct, D], f32r, tag="g")
                nc.gpsimd.dma_gather(
                    g[:],
                    embed_d[:].bitcast(f32r),
                    idx_t[:, row0 // 16 : (row0 + ch) // 16],
                    ch,
                    ch,
                    D,
                )
                gtiles.append(g)
                row0 += ch

            # Large centroid load only now, so it overlaps the gather window.
            cent_t = cpool.tile([P, D], f32)
            nc.scalar.dma_start(out=cent_t[:], in_=cent_d[:])

            # One-hot + count matmuls for every tile up front (no gather dep).
            ohs = []
            for t in range(T):
                oh = ohpool.tile([P, P], f32r, tag="oh")
                nc.vector.tensor_scalar(
                    out=oh[:],
                    in0=iota_t[:],
                    scalar1=yloc_t[:, t : t + 1],
                    scalar2=None,
                    op0=mybir.AluOpType.is_equal,
                )
                nc.tensor.matmul(
                    pcnt[:],
                    lhsT=oh[:],
                    rhs=ones_t[:],
                    start=(t == 0),
                    stop=(t == T - 1),
                )
                ohs.append(oh)

            # inv = (1-THETA) / (counts + EPS) -- ready long before data matmuls end
            inv = cpool.tile([P, 1], f32)
            nc.vector.tensor_scalar(
                out=inv[:],
                in0=pcnt[:, :1],
                scalar1=float(EPS),
                scalar2=None,
                op0=mybir.AluOpType.add,
            )
            nc.vector.reciprocal(inv[:], inv[:])
            nc.vector.tensor_scalar_mul(inv[:], inv[:], float(1.0 - THETA))

            # cents = THETA * centroid (off critical path)
            cents = cpool.tile([P, D], f32)
            nc.vector.tensor_scalar_mul(cents[:], cent_t[:], float(THETA))

            # Data matmuls, chunk by chunk as gathers land.
            t = 0
            for g, ct in zip(gtiles, chunks):
                for j in range(ct):
                    st, sp = (t == 0), (t == T - 1)
                    ohr = ohs[t][:]
                    nc.tensor.matmul(
                        ps0[:], lhsT=ohr, rhs=g[:, j, 0:512],
                        start=st, stop=sp,
                    )
                    nc.tensor.matmul(
                        ps1[:], lhsT=ohr, rhs=g[:, j, 512:D],
                        start=st, stop=sp,
                    )
                    t += 1

            # Tail: res = ps * inv + cents per 512-column half; store each
            # half as soon as it is ready so DMA overlaps the other half's
            # compute. All 128 partitions (pad rows are finite garbage,
            # sliced off on the host).
            res = cpool.tile([P, D], f32)
            for h, ps in ((0, ps0), (1, ps1)):
                lo, hi = h * 512, (h + 1) * 512
                nc.vector.tensor_scalar(
                    out=res[:, lo:hi],
                    in0=ps[:],
                    scalar1=inv[:, :1],
                    scalar2=None,
                    op0=mybir.AluOpType.mult,
                )
                nc.vector.tensor_add(
                    res[:, lo:hi], res[:, lo:hi], cents[:, lo:hi]
                )
                nc.scalar.dma_start(out=out_d[h], in_=res[:, lo:hi])

    nc.compile()
    return nc


def _shard_inputs(embed: np.ndarray, y: np.ndarray, centroid: np.ndarray):
    """Pure index-side sharding: assign each batch row to its class-owner core."""
    y64 = np.asarray(y).astype(np.int64).ravel()
    owner = y64 // CPC
    order = np.argsort(owner, kind="stable")
    counts = np.bincount(owner, minlength=NCORES)
    n_pad = max(int(-(-counts.max() // P)) * P, P)

    in_maps = []
    start = 0
    T = n_pad // P
    iota = np.broadcast_to(np.arange(P, dtype=np.float32), (P, P)).copy()
    for i in range(NCORES):
        n_i = int(counts[i])
        rows_i = order[start : start + n_i]
        start += n_i
        rows = np.zeros(n_pad, dtype=np.int16)
        rows[:n_i] = rows_i.astype(np.int16)
        yloc = np.full(n_pad, DUMMY, dtype=np.float32)
        yloc[:n_i] = (y64[rows_i] - i * CPC).astype(np.float32)
        # dma_gather idx layout: idx j at [j % 16, j // 16], replicated into
        # all 8 groups of 16 partitions (one copy per gpsimd Q7 core)
        idx_pt = np.tile(rows.reshape(n_pad // 16, 16).T, (8, 1))
        # yloc SBUF layout [128, T]: partition p, col t  <-  flat index t*128+p
        yloc_pt = np.ascontiguousarray(yloc.reshape(T, P).T)
        cent_pad = np.zeros((P, EMBED := centroid.shape[1]), dtype=np.float32)
        cent_pad[:CPC] = centroid[i * CPC : (i + 1) * CPC]
        in_maps.append(
            {
                "embed": embed,
                "idx": idx_pt,
                "yloc": yloc_pt,
                "cent": cent_pad,
                "iotac": iota,
            }
        )
    return in_maps, n_pad


def kernel(embed: np.ndarray, y: np.ndarray, centroid: np.ndarray) -> np.ndarray:
    global LAST_RESULTS
    embed = np.ascontiguousarray(np.asarray(embed, dtype=np.float32))
    centroid = np.ascontiguousarray(np.asarray(centroid, dtype=np.float32))

    in_maps, n_pad = _shard_inputs(embed, y, centroid)
    if n_pad not in _NC_CACHE:
        _NC_CACHE[n_pad] = _build_nc(n_pad)
    nc = _NC_CACHE[n_pad]

    trace = os.environ.get("KERNEL_TRACE", "0") == "1"
    res = run_bass_kernel_spmd(
        nc, in_maps, core_ids=list(range(NCORES)), trace=trace
    )
    LAST_RESULTS = res
    out = np.concatenate(
        [
            np.concatenate(
                [res.results[i]["out"][0][:CPC], res.results[i]["out"][1][:CPC]],
                axis=1,
            )
            for i in range(NCORES)
        ],
        axis=0,
    )
    return out.astype(np.float32)


# revision 33
# speedup vs baseline: 1.0790x; 1.0790x over previous
"""Trainium2 Bass kernel for nn_Centroid (segment_reduce + EMA).

Computes, for full inputs:
    sums   = segment_sum(embed, y, C)            # [C, D]
    counts = segment_sum(ones,  y, C)            # [C]
    out    = THETA*centroid + (1-THETA) * sums/(counts+EPS)

Sharding strategy (class-sharded, not batch-sharded):
  Core i owns classes [i*125, (i+1)*125). Host computes, per core, the list
  of batch-row indices whose label is owned by that core (pure index logic).
  Each core then:
    1. gathers its ~B/8 embed rows from HBM via chunked dma_gather (each
       full row is read exactly once across all cores -> same HBM traffic
       as a contiguous batch shard),
    2. builds a local one-hot [128 rows x 128 local classes] per k-tile via
       a host-provided iota constant + is_equal,
    3. matmul-accumulates sums [125,1024] and counts [125,2] in PSUM using
       float32r (full-rate fp32 matmul, TF32-like) straight from the
       gathered fp32 data (no bf16 cast stage),
    4. divides by counts, applies the EMA with its centroid slice, and
       writes its 125-row slice of the output.
  No cross-core reduction is needed at all (each class is computed fully on
  one core), so there are no collectives.

Scheduling notes (from trace analysis of the previous version):
  - The gather is SWDGE descriptor-generation / SDMA bound (~436 GB/s
    aggregate); the critical path is gpsimd desc-gen back-to-back.
  - The first gather must not wait on unrelated input DMAs: idx is loaded
    first (sync engine), iota/yloc on the scalar engine, and the large
    centroid load is issued only AFTER all gathers, so it overlaps the
    gather window instead of gating it.
  - All 17 one-hot builds + count matmuls are hoisted before the data
    matmuls (they need no gathered data), so counts/inv are ready long
    before the last data matmul and the tail is just mult,mult,add + store.
  - The output store is split by rows across both HWDGE engines.
"""

import os

import numpy as np

import concourse.bacc as bacc
import concourse.mybir as mybir
import concourse.tile as tile
from concourse import library_config
from concourse.bass_utils import run_bass_kernel_spmd
from concourse.tile_rust import add_dep_helper

NCORES = 8
B = 16384
C = 1000
D = 1024
CPC = C // NCORES  # classes per core = 125
P = 128
THETA = 0.7
EPS = 1e-8
DUMMY = CPC  # local class id used for padding rows; discarded

_NC_CACHE: dict[int, object] = {}

# test.py sets KERNEL_TRACE=1 to collect an NTFF profile; results stashed here.
LAST_RESULTS = None


def _chunk_plan(T: int) -> list[int]:
    """Gather chunk sizes in 128-row tiles.

    SWDGE is store-and-forward per call and the ring drains serialize, so the
    schedule is: gen1 | drain1+gen2 | drain2+gen3 | ... | drain_last | matmul
    tail. A small first chunk starts the drain chain early; small last chunks
    keep the trailing drain+matmul short; big middle chunks amortize the
    ~1.1us fixed desc-gen cost per call.
    """
    if T == 17:
        return [4, 3, 3, 3, 2, 1, 1]  # measured best (fewer calls -> less
        # per-call overhead and fewer straggler-queue boundaries)
    if T <= 4:
        return [1] * T
    chunks = [2]
    left = T - 2
    while left > 9:
        chunks.append(6)
        left -= 6
    if left > 3:
        chunks.append(left - 3)
        left = 3
    chunks += [2, 1]
    return chunks


def _build_nc(n_pad: int):
    """Build + compile the per-core Bass program for a padded shard of n_pad rows."""
    f32 = mybir.dt.float32
    f32r = mybir.dt.float32r
    i16 = mybir.dt.int16
    T = n_pad // P  # number of 128-row k-tiles
    chunks = _chunk_plan(T)

    nc = bacc.Bacc(
        "TRN2",
        target_bir_lowering=False,
        debug=False,
        enable_asserts=False,
        num_devices=NCORES,
    )
    embed_d = nc.dram_tensor("embed", [B, D], f32, kind="ExternalInput")
    idx_d = nc.dram_tensor("idx", [P, n_pad // 16], i16, kind="ExternalInput")
    yloc_d = nc.dram_tensor("yloc", [P, T], f32, kind="ExternalInput")
    # cent/out padded to 128 partitions: 128-partition HWDGE DMAs spread
    # across all 16 SDMA queues; odd partition counts can land on one queue.
    cent_d = nc.dram_tensor("cent", [P, D], f32, kind="ExternalInput")
    iota_d = nc.dram_tensor("iotac", [P, P], f32, kind="ExternalInput")
    pcol_d = nc.dram_tensor("pcol", [P, 1], f32, kind="ExternalInput")
    # out stacked as two contiguous [P, 512] halves so each half can be
    # stored as soon as its PSUM bank is done (host concatenates columns).
    out_d = nc.dram_tensor("out", [2, P, 512], f32, kind="ExternalOutput")

    # Trigger the Q7 mlp ucode load as early as possible (main block): the
    # load is asynchronous and the first DMAGatherAnt stalls ~10us on it.
    nc.gpsimd.load_library(library_config.mlp)

    with tile.TileContext(nc) as tc:
        with (
            tc.tile_pool(name="const", bufs=1) as cpool,
            tc.tile_pool(name="gather", bufs=len(chunks)) as gpool,
            tc.tile_pool(name="oh", bufs=T) as ohpool,
            tc.tile_pool(name="psum", bufs=1, space="PSUM") as psum,
        ):
            # No explicit load_library: the framework auto-inserts the mlp
            # reload for DMAGatherAnt; an explicit one doubles the ~6us Q7
            # ucode load on the critical path.

            # idx first (gates the gathers), small constants on the other
            # HWDGE engine so they cannot delay it.
            idx_t = cpool.tile([P, n_pad // 16], i16)
            nc.sync.dma_start(out=idx_t[:], in_=idx_d[:])
            iota_t = cpool.tile([P, P], f32)
            nc.scalar.dma_start(out=iota_t[:], in_=iota_d[:])
            yloc_t = cpool.tile([P, T], f32)
            nc.scalar.dma_start(out=yloc_t[:], in_=yloc_d[:])
            pcol_t = cpool.tile([P, 1], f32)
            nc.scalar.dma_start(out=pcol_t[:], in_=pcol_d[:])
            # memset cannot write float32r; synthesize 1.0s via DVE (rounds to f32r)
            ones_t = cpool.tile([P, 2], f32r)
            nc.vector.tensor_scalar(
                out=ones_t[:],
                in0=iota_t[:, 0:2],
                scalar1=0.0,
                scalar2=1.0,
                op0=mybir.AluOpType.mult,
                op1=mybir.AluOpType.add,
            )

            ps0 = psum.tile([P, 512], f32)
            ps1 = psum.tile([P, 512], f32)
            pcnt = psum.tile([P, 2], f32)

            # Issue every gather immediately; the only data dep is idx_t.
            gtiles = []
            row0 = 0
            for ct in chunks:
                ch = ct * P
                g = gpool.tile([P, ct, D], f32r, tag="g")
                nc.gpsimd.dma_gather(
                    g[:],
                    embed_d[:].bitcast(f32r),
                    idx_t[:, row0 // 16 : (row0 + ch) // 16],
                    ch,
                    ch,
                    D,
                )
                gtiles.append(g)
                row0 += ch

            # Large centroid load only now, so it overlaps the gather window.
            # Loaded as f32r: it feeds the diag(g) matmul below.
            cent_t = cpool.tile([P, D], f32r)
            nc.scalar.dma_start(out=cent_t[:], in_=cent_d[:].bitcast(f32r))

            # One-hot + count matmuls for every tile up front (no gather dep).
            ohs = []
            for t in range(T):
                oh = ohpool.tile([P, P], f32r, tag="oh")
                nc.vector.tensor_scalar(
                    out=oh[:],
                    in0=iota_t[:],
                    scalar1=yloc_t[:, t : t + 1],
                    scalar2=None,
                    op0=mybir.AluOpType.is_equal,
                )
                nc.tensor.matmul(
                    pcnt[:],
                    lhsT=oh[:],
                    rhs=ones_t[:],
                    start=(t == 0),
                    stop=(t == T - 1),
                )
                ohs.append(oh)

            # inv = (1-THETA) / (counts + EPS) -- ready long before data matmuls end
            inv = cpool.tile([P, 1], f32)
            nc.vector.tensor_scalar(
                out=inv[:],
                in0=pcnt[:, :1],
                scalar1=float(EPS),
                scalar2=None,
                op0=mybir.AluOpType.add,
            )
            nc.vector.reciprocal(inv[:], inv[:])
            nc.vector.tensor_scalar_mul(inv[:], inv[:], float(1.0 - THETA))

            # EMA via PSUM: accumulate diag(g) @ cent into ps0/ps1 where
            # g = (THETA/(1-THETA)) * (counts + EPS); after the final
            # multiply by inv = (1-THETA)/(counts+EPS) the counts factor
            # cancels, leaving exactly THETA*centroid. This removes the
            # tensor_add from the serial tail.
            gvec = cpool.tile([P, 1], f32)
            nc.vector.tensor_scalar(
                out=gvec[:],
                in0=pcnt[:, :1],
                scalar1=float(EPS),
                scalar2=float(THETA / (1.0 - THETA)),
                op0=mybir.AluOpType.add,
                op1=mybir.AluOpType.mult,
            )
            # eye_g[p, q] = (q == p) * g[p]  (one DVE op, f32r-rounded)
            eye_g = cpool.tile([P, P], f32r)
            nc.vector.tensor_scalar(
                out=eye_g[:],
                in0=iota_t[:],
                scalar1=pcol_t[:, :1],
                scalar2=gvec[:, :1],
                op0=mybir.AluOpType.is_equal,
                op1=mybir.AluOpType.mult,
            )

            # Data matmuls, chunk by chunk as gathers land.
            t = 0
            for g, ct in zip(gtiles, chunks):
                for j in range(ct):
                    st = t == 0
                    ohr = ohs[t][:]
                    nc.tensor.matmul(
                        ps0[:], lhsT=ohr, rhs=g[:, j, 0:512],
                        start=st, stop=False,
                    )
                    nc.tensor.matmul(
                        ps1[:], lhsT=ohr, rhs=g[:, j, 512:D],
                        start=st, stop=False,
                    )
                    t += 1
            # Fold THETA*centroid into PSUM (closes both accumulation groups).
            nc.tensor.matmul(
                ps0[:], lhsT=eye_g[:], rhs=cent_t[:, 0:512], start=False, stop=True
            )
            nc.tensor.matmul(
                ps1[:], lhsT=eye_g[:], rhs=cent_t[:, 512:D], start=False, stop=True
            )

            # Tail: res = ps * inv per 512-column half; store each half as
            # soon as it is ready so DMA overlaps the other half's compute.
            # All 128 partitions (pad rows are finite garbage, sliced off on
            # the host).
            res = cpool.tile([P, D], f32)
            for h, ps in ((0, ps0), (1, ps1)):
                lo, hi = h * 512, (h + 1) * 512
                nc.vector.tensor_scalar(
                    out=res[:, lo:hi],
                    in0=ps[:],
                    scalar1=inv[:, :1],
                    scalar2=None,
                    op0=mybir.AluOpType.mult,
                )
                nc.scalar.dma_start(out=out_d[h], in_=res[:, lo:hi])

    nc.compile()
    return nc


def _shard_inputs(embed: np.ndarray, y: np.ndarray, centroid: np.ndarray):
    """Pure index-side sharding: assign each batch row to its class-owner core."""
    y64 = np.asarray(y).astype(np.int64).ravel()
    owner = y64 // CPC
    order = np.argsort(owner, kind="stable")
    counts = np.bincount(owner, minlength=NCORES)
    n_pad = max(int(-(-counts.max() // P)) * P, P)

    in_maps = []
    start = 0
    T = n_pad // P
    iota = np.broadcast_to(np.arange(P, dtype=np.float32), (P, P)).copy()
    for i in range(NCORES):
        n_i = int(counts[i])
        rows_i = order[start : start + n_i]
        start += n_i
        rows = np.zeros(n_pad, dtype=np.int16)
        rows[:n_i] = rows_i.astype(np.int16)
        yloc = np.full(n_pad, DUMMY, dtype=np.float32)
        yloc[:n_i] = (y64[rows_i] - i * CPC).astype(np.float32)
        # dma_gather idx layout: idx j at [j % 16, j // 16], replicated into
        # all 8 groups of 16 partitions (one copy per gpsimd Q7 core)
        idx_pt = np.tile(rows.reshape(n_pad // 16, 16).T, (8, 1))
        # yloc SBUF layout [128, T]: partition p, col t  <-  flat index t*128+p
        yloc_pt = np.ascontiguousarray(yloc.reshape(T, P).T)
        cent_pad = np.zeros((P, EMBED := centroid.shape[1]), dtype=np.float32)
        cent_pad[:CPC] = centroid[i * CPC : (i + 1) * CPC]
        in_maps.append(
            {
                "embed": embed,
                "idx": idx_pt,
                "yloc": yloc_pt,
                "cent": cent_pad,
                "iotac": iota,
                "pcol": np.arange(P, dtype=np.float32).reshape(P, 1),
            }
        )
    return in_maps, n_pad


def kernel(embed: np.ndarray, y: np.ndarray, centroid: np.ndarray) -> np.ndarray:
    global LAST_RESULTS
    embed = np.ascontiguousarray(np.asarray(embed, dtype=np.float32))
    centroid = np.ascontiguousarray(np.asarray(centroid, dtype=np.float32))

    in_maps, n_pad = _shard_inputs(embed, y, centroid)
    if n_pad not in _NC_CACHE:
        _NC_CACHE[n_pad] = _build_nc(n_pad)
    nc = _NC_CACHE[n_pad]

    trace = os.environ.get("KERNEL_TRACE", "0") == "1"
    res = run_bass_kernel_spmd(
        nc, in_maps, core_ids=list(range(NCORES)), trace=trace
    )
    LAST_RESULTS = res
    out = np.concatenate(
        [
            np.concatenate(
                [res.results[i]["out"][0][:CPC], res.results[i]["out"][1][:CPC]],
                axis=1,
            )
            for i in range(NCORES)
        ],
        axis=0,
    )
    return out.astype(np.float32)
